# revision 2
# baseline (speedup 1.0000x reference)
"""Trainium2 Bass kernel for the KnowledgeGraphEmbedding loss.

Computes, for P=1024 relations sharded 128-per-core across 8 NeuronCores:
    li = Lp_w[p] @ wi          (wi = tag_rep[tag1_idx])
    rj = Rp_w[p] @ wj          (wj = tag_rep[tag2_idx])
    dist[p] = sum_h (li - rj)^2
    out = [dist*rel, dist*(1-rel), rel, 1-rel]   (rel in {0,1})

Memory-bound: all of Lp_w/Rp_w streams from HBM once, quantized host-side
to fp8 e4m3 (ALPHA=256 weights, BETA=32 for wi/-wj; end-to-end max rel err
~1e-2 vs the 2e-2 gate).

Device dataflow (per core, 128 relations):
  - Moving operand: one fused [125, 12288] fp8 transfer per batch b
    (15 batches), row k = (m=k//25, i=k%25), col = s*6144 + c*1024 +
    j*512 + hg*128 + rel.  No pad rows: contraction K=125.
  - Stationary: block-diagonal wi/-wj chunks [125, 2, 5] (DoubleRow,
    2 fp8/cell), 12 per batch (2 sides x 6 e-chunks of 50), accumulating
    diff = li - rj for 20 h-rows x 128 rel in one PSUM bank [5, 512].
  - Evac: ScalarE activation Square(scale*ps) -> sq [5, 512] f32, then a
    small DMA of sq to HBM per batch.  Host sums the 5*4*15 = 300 squared
    diffs per relation and applies the rel masks (cheap unshard-side work).
  - Warm-up: a memset tile feeds 12 dummy N=512 matmuls right after the
    preamble so the PE HAM clock-gate hits 8/8 before the first real batch.
  - The last batch is split into 3 piece-DMAs (L side / R c0-3 / R c4-5)
    so its matmuls start before the final bytes land, shortening the tail.
"""

from contextlib import ExitStack

import numpy as np

N_CORES = 8
P_TOTAL = 1024
H = 300
E = 300
P_LOC = P_TOTAL // N_CORES  # 128 relations per core

G = 25        # e-fragment length per block-diag group per plane
M = 5         # h-rows packed per column set (stationary free dim)
J = 2         # DoubleRow planes
K = G * M     # 125 contraction rows (no padding)
NCH = E // (G * J)  # 6 e-chunks of 50
HG = H // M   # 60 h-groups
HB = 4        # h-groups per batch
NB = HG // HB  # 15 batches
NCOL = HB * P_LOC  # 512 output columns per matmul (= one PSUM bank f32)
CW = J * NCOL      # 1024 moving columns per (side, chunk)
ROWB = 2 * NCH * CW  # 12288 fused row bytes per batch
STP = 16      # stationary plane stride (DoubleRow wants step % 16 == 0)

ALPHA = 256.0  # weight scale into e4m3 range
BETA = 32.0    # wi/wj scale into e4m3 range
INV_SCALE = 1.0 / (ALPHA * BETA)
F8_MAX = 240.0

# Set by test harness to capture a profile; kernel() stores results here.
TRACE = False
LAST_RESULT = None

_CACHE: dict = {}


def _build_nc():
    import concourse.bacc as bacc
    import concourse.mybir as mybir
    import concourse.tile as tile

    f32 = mybir.dt.float32
    f8 = mybir.dt.float8e4

    nc = bacc.Bacc("TRN2", debug=False)

    # Fused moving operand: row b*K+k, col s*6144 + c*1024 + j*512 + hg*128 + rel
    #   = Wq_s[rel, (b*HB+hg)*M + k//G, c*G*J + j*G + k%G]
    x = nc.dram_tensor("x", [NB * K, ROWB], f8, kind="ExternalInput").ap()
    # Block-diagonal stationaries: [125, (s*NCH+c)*2*STP + j*STP + m]
    st = nc.dram_tensor("st", [K, 2 * NCH * J * STP], f8, kind="ExternalInput").ap()
    # Per-batch squared diffs: [5, b*512 + hg*128 + rel]
    outd = nc.dram_tensor("outd", [M, NB * NCOL], f32, kind="ExternalOutput").ap()

    with tile.TileContext(nc) as tc, ExitStack() as ctx:
        const_pool = ctx.enter_context(tc.tile_pool(name="const", bufs=1))
        xpool = ctx.enter_context(tc.tile_pool(name="xmov", bufs=8))
        sqpool = ctx.enter_context(tc.tile_pool(name="sq", bufs=3))
        psum_pool = ctx.enter_context(tc.tile_pool(name="ps", bufs=4, space="PSUM"))
        warm_pool = ctx.enter_context(tc.tile_pool(name="wps", bufs=1, space="PSUM"))

        st_sb = const_pool.tile([K, 2 * NCH * J * STP], f8)
        nc.scalar.dma_start(st_sb[:], st[:])

        # HAM pre-warm: a zeroed tile feeds dummy matmuls that only depend on
        # the DVE memset, so PE activity starts right after the preamble and
        # the clock-gate reaches 8/8 before the first real batch lands.
        wtile = const_pool.tile([128, 512], f8)
        nc.vector.memset(wtile[:], 0)
        warm_ps = warm_pool.tile([M, 512], f32)
        for _ in range(12):
            nc.tensor.matmul(
                warm_ps[:], wtile[:, 0:M], wtile[:, :], start=True, stop=True
            )

        for b in range(NB):
            last = b == NB - 1
            if not last:
                xt = xpool.tile([K, ROWB], f8, name="xt", tag="xt")
                nc.sync.dma_start(xt[:], x[b * K : (b + 1) * K, :])
                mov = lambda idx: xt[:, idx * CW : (idx + 1) * CW]
            else:
                # Tail split: matmuls start as each piece lands.
                xa = const_pool.tile([K, 6 * CW], f8)
                xb = const_pool.tile([K, 4 * CW], f8)
                xc = const_pool.tile([K, 2 * CW], f8)
                r0 = b * K
                nc.sync.dma_start(xa[:], x[r0 : r0 + K, 0 : 6 * CW])
                nc.sync.dma_start(xb[:], x[r0 : r0 + K, 6 * CW : 10 * CW])
                nc.sync.dma_start(xc[:], x[r0 : r0 + K, 10 * CW : 12 * CW])

                def mov(idx):
                    if idx < 6:
                        return xa[:, idx * CW : (idx + 1) * CW]
                    if idx < 10:
                        return xb[:, (idx - 6) * CW : (idx - 5) * CW]
                    return xc[:, (idx - 10) * CW : (idx - 9) * CW]

            ps = psum_pool.tile([M, NCOL], f32, name="ps", tag="ps")
            for n in range(2 * NCH):
                base = n * J * STP
                lhsT = st_sb[:, base : base + J * STP].rearrange(
                    "p (j x) -> p j x", j=J
                )[:, :, 0:M]
                nc.tensor.matmul(
                    ps[:],
                    lhsT,
                    mov(n).rearrange("p (j n) -> p j n", j=J),
                    start=(n == 0),
                    stop=(n == 2 * NCH - 1),
                    perf_mode=mybir.MatmulPerfMode.DoubleRow,
                )

            sq = sqpool.tile([M, NCOL], f32, name="sq", tag="sq")
            nc.scalar.activation(
                sq[:], ps[:], mybir.ActivationFunctionType.Square, scale=INV_SCALE
            )
            nc.scalar.dma_start(outd[:, b * NCOL : (b + 1) * NCOL], sq[:])

    nc.compile()
    return nc


def _quant_f8(x):
    import ml_dtypes

    return np.clip(x, -F8_MAX, F8_MAX).astype(ml_dtypes.float8_e4m3)


def _prepack(lq_core, rq_core):
    """2x [128, 300, 300] e4m3 -> [NB*K, ROWB] fused moving layout."""
    parts = []
    for wq in (lq_core, rq_core):
        a = wq.reshape(P_LOC, NB, HB, M, NCH, J, G)  # rel, b, hg, m, c, j, i
        a = a.transpose(1, 3, 6, 4, 5, 2, 0)  # b, m, i, c, j, hg, rel
        parts.append(np.ascontiguousarray(a).reshape(NB, K, NCH * CW))
    return np.ascontiguousarray(np.concatenate(parts, axis=2)).reshape(NB * K, ROWB)


def kernel(tag_rep, Lp_w, Rp_w, relation, tag1_idx, tag2_idx):
    global LAST_RESULT
    from concourse.bass_utils import run_bass_kernel_spmd

    if "nc" not in _CACHE:
        _CACHE["nc"] = _build_nc()
    nc = _CACHE["nc"]

    tag_rep = np.asarray(tag_rep)
    rel = np.asarray(relation).astype(np.float32)  # values in {0, 1}

    wi = np.asarray(tag_rep[int(tag1_idx)], dtype=np.float32)
    wj = np.asarray(tag_rep[int(tag2_idx)], dtype=np.float32)
    v_l = _quant_f8(wi * BETA)
    v_r = _quant_f8(-wj * BETA)

    st = np.zeros((K, 2 * NCH * J * STP), dtype=v_l.dtype)
    for s, v in ((0, v_l), (1, v_r)):
        for c in range(NCH):
            for j in range(J):
                for m in range(M):
                    e0 = c * G * J + j * G
                    st[m * G : (m + 1) * G, (s * NCH + c) * J * STP + j * STP + m] = v[
                        e0 : e0 + G
                    ]

    lq = _quant_f8(np.asarray(Lp_w) * ALPHA)
    rq = _quant_f8(np.asarray(Rp_w) * ALPHA)

    in_maps = []
    for core in range(N_CORES):
        sl = slice(core * P_LOC, (core + 1) * P_LOC)
        in_maps.append({"x": _prepack(lq[sl], rq[sl]), "st": st})

    kw = {}
    if TRACE:
        kw = dict(trace=True, trace_cores=[0])
    res = run_bass_kernel_spmd(nc, in_maps, core_ids=list(range(N_CORES)), **kw)
    LAST_RESULT = res

    out_full = np.empty((4, P_TOTAL), dtype=np.float32)
    for core in range(N_CORES):
        sq = res.results[core]["outd"]  # [5, NB*512] true squared diffs
        dist = sq.reshape(M, NB, HB, P_LOC).sum(axis=(0, 1, 2))
        cs = slice(core * P_LOC, (core + 1) * P_LOC)
        rel_c = rel[cs]
        out_full[0, cs] = dist * rel_c
        out_full[1, cs] = dist * (1.0 - rel_c)
        out_full[2, cs] = rel_c
        out_full[3, cs] = 1.0 - rel_c
    return out_full


# revision 3
# speedup vs baseline: 2.3003x; 2.3003x over previous
"""Trainium2 Bass kernel for the KnowledgeGraphEmbedding loss.

Computes, for P=1024 relations sharded 128-per-core across 8 NeuronCores:
    li = Lp_w[p] @ wi          (wi = tag_rep[tag1_idx])
    rj = Rp_w[p] @ wj          (wj = tag_rep[tag2_idx])
    dist[p] = sum_h (li - rj)^2
    out = [dist*rel, dist*(1-rel), rel, 1-rel]   (rel in {0,1})

Memory-bound: all of Lp_w/Rp_w streams from HBM once, quantized host-side
to fp8 e4m3 (ALPHA=256 weights, BETA=32 for wi/-wj; end-to-end max rel err
~1e-2 vs the 2e-2 gate).

Device dataflow (per core, 128 relations):
  - Moving operand: one fused [125, 12288] fp8 transfer per batch b
    (15 batches), row k = (m=k//25, i=k%25), col = s*6144 + c*1024 +
    j*512 + hg*128 + rel.  No pad rows: contraction K=125.
  - Stationary: block-diagonal wi/-wj chunks [125, 2, 5] (DoubleRow,
    2 fp8/cell), 12 per batch (2 sides x 6 e-chunks of 50), accumulating
    diff = li - rj for 20 h-rows x 128 rel in one PSUM bank [5, 512].
  - Evac: ScalarE activation Square(scale*ps) -> sq [5, 512] f32, then a
    small DMA of sq to HBM per batch.  Host sums the 5*4*15 = 300 squared
    diffs per relation and applies the rel masks (cheap unshard-side work).
  - Warm-up: a memset tile feeds 12 dummy N=512 matmuls right after the
    preamble so the PE HAM clock-gate hits 8/8 before the first real batch.
  - The last batch is split into 3 piece-DMAs (L side / R c0-3 / R c4-5)
    so its matmuls start before the final bytes land, shortening the tail.
"""

from contextlib import ExitStack

import numpy as np

N_CORES = 8
P_TOTAL = 1024
H = 300
E = 300
P_LOC = P_TOTAL // N_CORES  # 128 relations per core

G = 25        # e-fragment length per block-diag group per plane
M = 5         # h-rows packed per column set (stationary free dim)
J = 2         # DoubleRow planes
KU = G * M    # 125 useful contraction rows
K = 128       # padded rows: the 128-partition DMA spray needs all 16 engines
NCH = E // (G * J)  # 6 e-chunks of 50
HG = H // M   # 60 h-groups
HB = 4        # h-groups per batch
NB = HG // HB  # 15 batches
NCOL = HB * P_LOC  # 512 output columns per matmul (= one PSUM bank f32)
CW = J * NCOL      # 1024 moving columns per (side, chunk)
ROWB = 2 * NCH * CW  # 12288 fused row bytes per batch
STP = 16      # stationary plane stride (DoubleRow wants step % 16 == 0)

ALPHA = 256.0  # weight scale into e4m3 range
BETA = 32.0    # wi/wj scale into e4m3 range
INV_SCALE = 1.0 / (ALPHA * BETA)
F8_MAX = 240.0

# Set by test harness to capture a profile; kernel() stores results here.
TRACE = False
LAST_RESULT = None

_CACHE: dict = {}


def _build_nc():
    import concourse.bacc as bacc
    import concourse.mybir as mybir
    import concourse.tile as tile

    f32 = mybir.dt.float32
    f8 = mybir.dt.float8e4

    nc = bacc.Bacc("TRN2", debug=False)

    # Fused moving operand: row b*K+k, col s*6144 + c*1024 + j*512 + hg*128 + rel
    #   = Wq_s[rel, (b*HB+hg)*M + k//G, c*G*J + j*G + k%G]
    x = nc.dram_tensor("x", [NB * K, ROWB], f8, kind="ExternalInput").ap()
    # Block-diagonal stationaries: [125, (s*NCH+c)*2*STP + j*STP + m]
    st = nc.dram_tensor("st", [K, 2 * NCH * J * STP], f8, kind="ExternalInput").ap()
    # Per-batch squared diffs: [5, b*512 + hg*128 + rel]
    outd = nc.dram_tensor("outd", [M, NB * NCOL], f32, kind="ExternalOutput").ap()

    with tile.TileContext(nc) as tc, ExitStack() as ctx:
        const_pool = ctx.enter_context(tc.tile_pool(name="const", bufs=1))
        xpool = ctx.enter_context(tc.tile_pool(name="xmov", bufs=8))
        sqpool = ctx.enter_context(tc.tile_pool(name="sq", bufs=3))
        psum_pool = ctx.enter_context(tc.tile_pool(name="ps", bufs=4, space="PSUM"))
        warm_pool = ctx.enter_context(tc.tile_pool(name="wps", bufs=1, space="PSUM"))

        st_sb = const_pool.tile([K, 2 * NCH * J * STP], f8)
        nc.scalar.dma_start(st_sb[:], st[:])

        # HAM pre-warm: a zeroed tile feeds dummy matmuls that only depend on
        # the DVE memset, so PE activity starts right after the preamble and
        # the clock-gate reaches 8/8 before the first real batch lands.
        wtile = const_pool.tile([128, 512], f8)
        nc.vector.memset(wtile[:], 0)
        warm_ps = warm_pool.tile([M, 512], f32)
        for _ in range(12):
            nc.tensor.matmul(
                warm_ps[:], wtile[:, 0:M], wtile[:, :], start=True, stop=True
            )

        for b in range(NB):
            last = b == NB - 1
            if not last:
                xt = xpool.tile([K, ROWB], f8, name="xt", tag="xt")
                nc.sync.dma_start(xt[:], x[b * K : (b + 1) * K, :])
                mov = lambda idx: xt[:, idx * CW : (idx + 1) * CW]
            else:
                # Tail split: matmuls start as each piece lands.
                xa = const_pool.tile([K, 6 * CW], f8)
                xb = const_pool.tile([K, 4 * CW], f8)
                xc = const_pool.tile([K, 2 * CW], f8)
                r0 = b * K
                nc.sync.dma_start(xa[:], x[r0 : r0 + K, 0 : 6 * CW])
                nc.sync.dma_start(xb[:], x[r0 : r0 + K, 6 * CW : 10 * CW])
                nc.sync.dma_start(xc[:], x[r0 : r0 + K, 10 * CW : 12 * CW])

                def mov(idx):
                    if idx < 6:
                        return xa[:, idx * CW : (idx + 1) * CW]
                    if idx < 10:
                        return xb[:, (idx - 6) * CW : (idx - 5) * CW]
                    return xc[:, (idx - 10) * CW : (idx - 9) * CW]

            ps = psum_pool.tile([M, NCOL], f32, name="ps", tag="ps")
            for n in range(2 * NCH):
                base = n * J * STP
                lhsT = st_sb[:, base : base + J * STP].rearrange(
                    "p (j x) -> p j x", j=J
                )[:, :, 0:M]
                nc.tensor.matmul(
                    ps[:],
                    lhsT,
                    mov(n).rearrange("p (j n) -> p j n", j=J),
                    start=(n == 0),
                    stop=(n == 2 * NCH - 1),
                    perf_mode=mybir.MatmulPerfMode.DoubleRow,
                )

            sq = sqpool.tile([M, NCOL], f32, name="sq", tag="sq")
            nc.scalar.activation(
                sq[:], ps[:], mybir.ActivationFunctionType.Square, scale=INV_SCALE
            )
            nc.scalar.dma_start(outd[:, b * NCOL : (b + 1) * NCOL], sq[:])

    nc.compile()
    return nc


def _quant_f8(x):
    import ml_dtypes

    return np.clip(x, -F8_MAX, F8_MAX).astype(ml_dtypes.float8_e4m3)


def _prepack(lq_core, rq_core):
    """2x [128, 300, 300] e4m3 -> [NB*K, ROWB] fused moving layout, K-padded."""
    parts = []
    for wq in (lq_core, rq_core):
        a = wq.reshape(P_LOC, NB, HB, M, NCH, J, G)  # rel, b, hg, m, c, j, i
        a = a.transpose(1, 3, 6, 4, 5, 2, 0)  # b, m, i, c, j, hg, rel
        parts.append(np.ascontiguousarray(a).reshape(NB, KU, NCH * CW))
    x = np.zeros((NB, K, ROWB), dtype=lq_core.dtype)
    x[:, :KU] = np.concatenate(parts, axis=2)
    return x.reshape(NB * K, ROWB)


def kernel(tag_rep, Lp_w, Rp_w, relation, tag1_idx, tag2_idx):
    global LAST_RESULT
    from concourse.bass_utils import run_bass_kernel_spmd

    if "nc" not in _CACHE:
        _CACHE["nc"] = _build_nc()
    nc = _CACHE["nc"]

    tag_rep = np.asarray(tag_rep)
    rel = np.asarray(relation).astype(np.float32)  # values in {0, 1}

    wi = np.asarray(tag_rep[int(tag1_idx)], dtype=np.float32)
    wj = np.asarray(tag_rep[int(tag2_idx)], dtype=np.float32)
    v_l = _quant_f8(wi * BETA)
    v_r = _quant_f8(-wj * BETA)

    st = np.zeros((K, 2 * NCH * J * STP), dtype=v_l.dtype)
    for s, v in ((0, v_l), (1, v_r)):
        for c in range(NCH):
            for j in range(J):
                for m in range(M):
                    e0 = c * G * J + j * G
                    st[m * G : (m + 1) * G, (s * NCH + c) * J * STP + j * STP + m] = v[
                        e0 : e0 + G
                    ]

    lq = _quant_f8(np.asarray(Lp_w) * ALPHA)
    rq = _quant_f8(np.asarray(Rp_w) * ALPHA)

    in_maps = []
    for core in range(N_CORES):
        sl = slice(core * P_LOC, (core + 1) * P_LOC)
        in_maps.append({"x": _prepack(lq[sl], rq[sl]), "st": st})

    kw = {}
    if TRACE:
        kw = dict(trace=True, trace_cores=[0])
    res = run_bass_kernel_spmd(nc, in_maps, core_ids=list(range(N_CORES)), **kw)
    LAST_RESULT = res

    out_full = np.empty((4, P_TOTAL), dtype=np.float32)
    for core in range(N_CORES):
        sq = res.results[core]["outd"]  # [5, NB*512] true squared diffs
        dist = sq.reshape(M, NB, HB, P_LOC).sum(axis=(0, 1, 2))
        cs = slice(core * P_LOC, (core + 1) * P_LOC)
        rel_c = rel[cs]
        out_full[0, cs] = dist * rel_c
        out_full[1, cs] = dist * (1.0 - rel_c)
        out_full[2, cs] = rel_c
        out_full[3, cs] = 1.0 - rel_c
    return out_full
